# revision 2
# baseline (speedup 1.0000x reference)
"""2D Haar DWT (level 1) Trainium2 Bass kernel.

Input  x: [16, 64, 256, 256] f32
Output y: [16, 256, 128, 128] f32, y[n, s*64+c, i, j] = Haar mix s of the
2x2 block x[n, c, 2i:2i+2, 2j:2j+2].

Sharding: pure data parallel over the batch dim — core k gets batches
[2k, 2k+2).

Per-core kernel design (memory-bound problem, ~67 MB traffic/core):
  - Load groups of G=4 channel planes with DMA so SBUF partition p holds
    input row (r*128 + p)  ->  1 MB contiguous-ish loads (1 KB runs).
  - Vertical Haar butterfly on TensorE: one fp32 matmul per plane against a
    constant 128x128 block-butterfly matrix W (0.5 scale folded in).
    PSUM partition q<64 = 0.5*(row2q+row2q+1), q>=64 = 0.5*(row2q-row2q+1).
  - Horizontal butterfly on VectorE: two tensor_tensor ops (add/sub) with
    stride-2 PSUM reads produce all 4 subbands.
  - Stores: 4 DMA stores per group (one per subband), 256 KB each, 512 B runs.
"""

import sys

sys.path.insert(0, "/opt/trn_rl_repo")

import numpy as np

import concourse.bacc as bacc
import concourse.mybir as mybir
from concourse.tile import TileContext

N_CORES = 8
N_PER_CORE = 2  # batches per core
C = 64  # input channels
H = 256
W = 256
G = 4  # channels per group
F32 = mybir.dt.float32


def _butterfly_weights() -> np.ndarray:
    # lhsT for out = lhsT.T @ rhs: out[q] = sum_p Wb[p, q] * in[p].
    # q in [0,64): 0.5*(in[2q] + in[2q+1]); q in [64,128): 0.5*(in[2(q-64)] - in[2(q-64)+1])
    Wb = np.zeros((128, 128), np.float32)
    for k in range(64):
        Wb[2 * k, k] = 0.5
        Wb[2 * k + 1, k] = 0.5
        Wb[2 * k, 64 + k] = 0.5
        Wb[2 * k + 1, 64 + k] = -0.5
    return Wb


def build_nc():
    nc = bacc.Bacc("TRN2", target_bir_lowering=False, debug=False)
    x = nc.dram_tensor("x", [N_PER_CORE, C, H, W], F32, kind="ExternalInput")
    y = nc.dram_tensor("y", [N_PER_CORE, 4 * C, H // 2, W // 2], F32, kind="ExternalOutput")
    wdram = nc.inline_tensor(_butterfly_weights(), name="haar_w")

    with TileContext(nc) as tc:
        with (
            tc.tile_pool(name="wpool", bufs=1) as wpool,
            tc.tile_pool(name="inpool", bufs=3) as inpool,
            tc.tile_pool(name="outpool", bufs=3) as outpool,
            tc.tile_pool(name="psum", bufs=2, space="PSUM") as psump,
        ):
            wt = wpool.tile([128, 128], F32)
            nc.sync.dma_start(out=wt[:], in_=wdram[:])

            for n in range(N_PER_CORE):
                for c0 in range(0, C, G):
                    # --- load G planes: it[p, c, r, w] = x[n, c0+c, r*128+p, w]
                    it = inpool.tile([128, G * 512], F32, tag="in")
                    src = x[n, c0 : c0 + G].rearrange("c (r p) w -> p c r w", p=128)
                    nc.sync.dma_start(
                        out=it[:].rearrange("p (c r w) -> p c r w", c=G, r=2),
                        in_=src,
                    )

                    # --- vertical butterfly on PE, one matmul per plane
                    ps = psump.tile([128, G * 512], F32, tag="ps")
                    ps_c = ps[:].rearrange("q (c f) -> q c f", c=G)
                    it_c = it[:].rearrange("p (c f) -> p c f", c=G)
                    for c in range(G):
                        nc.tensor.matmul(
                            out=ps_c[:, c],
                            lhsT=wt[:],
                            rhs=it_c[:, c],
                            start=True,
                            stop=True,
                        )

                    # --- horizontal butterfly. DVE can read at most one PSUM
                    # operand per op, so ScalarE first deinterleaves the even
                    # columns into SBUF; each DVE op then reads one SBUF + one
                    # PSUM operand.
                    ps_j = ps[:].rearrange("q (c r j t) -> q c r j t", c=G, r=2, t=2)
                    ev = outpool.tile([128, G * 256], F32, tag="ev")
                    ev_v = ev[:].rearrange("q (c r j) -> q c r j", c=G, r=2)
                    nc.scalar.copy(out=ev_v, in_=ps_j[:, :, :, :, 0])

                    oa = outpool.tile([128, G * 256], F32, tag="oa")
                    ob = outpool.tile([128, G * 256], F32, tag="ob")
                    oa_v = oa[:].rearrange("q (c r j) -> q c r j", c=G, r=2)
                    ob_v = ob[:].rearrange("q (c r j) -> q c r j", c=G, r=2)
                    nc.vector.tensor_add(
                        out=oa_v, in0=ev_v, in1=ps_j[:, :, :, :, 1]
                    )
                    nc.vector.tensor_sub(
                        out=ob_v, in0=ev_v, in1=ps_j[:, :, :, :, 1]
                    )

                    # --- stores: (tile, partition base, subband)
                    for t_, ks, s in ((oa, 0, 0), (oa, 64, 1), (ob, 0, 2), (ob, 64, 3)):
                        dst = y[n, s * C + c0 : s * C + c0 + G].rearrange(
                            "c (r k) j -> k c r j", r=2
                        )
                        nc.scalar.dma_start(
                            out=dst,
                            in_=t_[ks : ks + 64].rearrange(
                                "k (c r j) -> k c r j", c=G, r=2
                            ),
                        )

    nc.finalize()
    return nc


_NC = None


def _get_nc():
    global _NC
    if _NC is None:
        _NC = build_nc()
    return _NC


def kernel(x: np.ndarray) -> np.ndarray:
    from concourse.bass_utils import run_bass_kernel_spmd

    x = np.ascontiguousarray(np.asarray(x), dtype=np.float32)
    assert x.shape == (16, C, H, W), x.shape

    nc = _get_nc()
    in_maps = [
        {"x": x[k * N_PER_CORE : (k + 1) * N_PER_CORE]} for k in range(N_CORES)
    ]
    res = run_bass_kernel_spmd(nc, in_maps, core_ids=list(range(N_CORES)))
    return np.concatenate([r["y"] for r in res.results], axis=0)


# revision 4
# speedup vs baseline: 1.0626x; 1.0626x over previous
"""2D Haar DWT (level 1) Trainium2 Bass kernel.

Input  x: [16, 64, 256, 256] f32
Output y: [16, 256, 128, 128] f32, y[n, s*64+c, i, j] = Haar mix s of the
2x2 block x[n, c, 2i:2i+2, 2j:2j+2].

Sharding: pure data parallel over the batch dim — core k gets batches
[2k, 2k+2).

Per-core design (memory-bound, ~67 MB HBM traffic/core, ~190 us roofline):

Oct-row layout: a group of G=4 channel planes (1 MB, contiguous in DRAM) is
loaded so SBUF partition p = (c*32 + row//8) holds 8 consecutive rows — a
pure [128, 2048] reshape of the DRAM stream (8 KB contiguous runs, 2-dim
AP). Both Haar butterfly stages are then same-partition, unit/2-strided
VectorE ops:
  stage 1 (vertical):  sum/diff of row pairs  -> one sd tile [128,2,4,256]
  scale: ScalarE in-place *0.5 on sd (folds the Haar normalization)
  stage 2 (horizontal): sd even +/- odd cols  -> oadd [p,v,rh4,j] holds
       subbands 0 (v=0) and 1 (v=1); osub holds subbands 2 and 3.
Stores are [128, 512] -> 256 KB per subband pair slice with 2 KB contiguous
DRAM runs (2-dim AP, full 128 partitions): output row i = 4*(p%32) + rh4.

Engine budget/core: DVE 4 ops/group ~150 us, ACT ~66 us, DMA ~195-215 us
(bottleneck), PE unused (fp32 matmul runs at 1/4 rate — measured slower
than DVE for this transform).
"""

import sys

sys.path.insert(0, "/opt/trn_rl_repo")

import numpy as np

import concourse.bacc as bacc
import concourse.mybir as mybir
from concourse.tile import TileContext

N_CORES = 8
N_PER_CORE = 2  # batches per core
C = 64  # input channels
H = 256
W = 256
G = 4  # channels per group (1 MB loads)
F32 = mybir.dt.float32


def build_nc():
    nc = bacc.Bacc("TRN2", target_bir_lowering=False, debug=False)
    x = nc.dram_tensor("x", [N_PER_CORE, C, H, W], F32, kind="ExternalInput")
    y = nc.dram_tensor("y", [N_PER_CORE, 4 * C, H // 2, W // 2], F32, kind="ExternalOutput")

    with TileContext(nc) as tc:
        with (
            tc.tile_pool(name="inpool", bufs=3) as inpool,
            tc.tile_pool(name="sdpool", bufs=3) as sdpool,
            tc.tile_pool(name="outpool", bufs=3) as outpool,
        ):
            gi = 0
            for n in range(N_PER_CORE):
                for c0 in range(0, C, G):
                    # --- load: pure reshape of the 1 MB contiguous group.
                    # it[p, o, w] = x[n, c0 + p//32, 8*(p%32) + o, w]
                    it = inpool.tile([128, 2048], F32, tag="in")
                    src = x[n, c0 : c0 + G].rearrange("c (q o) w -> (c q) o w", o=8)
                    nc.sync.dma_start(
                        out=it[:].rearrange("p (o w) -> p o w", o=8), in_=src
                    )

                    # --- stage 1 (vertical): rows 2t / 2t+1 within a partition
                    itv = it[:].rearrange("p (r t w) -> p r t w", r=4, t=2)
                    sd = sdpool.tile([128, 2 * 1024], F32, tag="sd")
                    sdv = sd[:].rearrange("p (v r w) -> p v r w", v=2, r=4)
                    nc.vector.tensor_add(
                        out=sdv[:, 0], in0=itv[:, :, 0, :], in1=itv[:, :, 1, :]
                    )
                    nc.vector.tensor_sub(
                        out=sdv[:, 1], in0=itv[:, :, 0, :], in1=itv[:, :, 1, :]
                    )

                    # --- Haar 0.5 normalization, in place on ScalarE
                    nc.scalar.mul(sd[:], sd[:], 0.5)

                    # --- stage 2 (horizontal): even/odd column butterfly
                    sdj = sd[:].rearrange("p (v r j t) -> p v r j t", v=2, r=4, t=2)
                    oadd = outpool.tile([128, 2 * 512], F32, tag="oadd")
                    osub = outpool.tile([128, 2 * 512], F32, tag="osub")
                    oadd_v = oadd[:].rearrange("p (v r j) -> p v r j", v=2, r=4)
                    osub_v = osub[:].rearrange("p (v r j) -> p v r j", v=2, r=4)
                    nc.vector.tensor_add(
                        out=oadd_v, in0=sdj[..., 0], in1=sdj[..., 1]
                    )
                    nc.vector.tensor_sub(
                        out=osub_v, in0=sdj[..., 0], in1=sdj[..., 1]
                    )

                    # --- stores: (tile, v, subband); i = 4*(p%32) + rh4, so the
                    # DRAM side is [p step 512][(r j) 512] — 2 KB runs.
                    for t_, v, s in ((oadd, 0, 0), (oadd, 1, 1), (osub, 0, 2), (osub, 1, 3)):
                        dst = y[n, s * C + c0 : s * C + c0 + G].rearrange(
                            "c (q r) j -> (c q) (r j)", r=4
                        )
                        eng = nc.sync if gi % 2 == 0 else nc.scalar
                        eng.dma_start(
                            out=dst,
                            in_=t_[:].rearrange("p (v f) -> p v f", v=2)[:, v],
                        )
                    gi += 1

    nc.finalize()
    return nc


_NC = None


def _get_nc():
    global _NC
    if _NC is None:
        _NC = build_nc()
    return _NC


def kernel(x: np.ndarray) -> np.ndarray:
    from concourse.bass_utils import run_bass_kernel_spmd

    x = np.ascontiguousarray(np.asarray(x), dtype=np.float32)
    assert x.shape == (16, C, H, W), x.shape

    nc = _get_nc()
    in_maps = [
        {"x": x[k * N_PER_CORE : (k + 1) * N_PER_CORE]} for k in range(N_CORES)
    ]
    res = run_bass_kernel_spmd(nc, in_maps, core_ids=list(range(N_CORES)))
    return np.concatenate([r["y"] for r in res.results], axis=0)


# revision 5
# speedup vs baseline: 1.1047x; 1.0395x over previous
"""2D Haar DWT (level 1) Trainium2 Bass kernel.

Input  x: [16, 64, 256, 256] f32
Output y: [16, 256, 128, 128] f32, y[n, s*64+c, i, j] = Haar mix s of the
2x2 block x[n, c, 2i:2i+2, 2j:2j+2].

Sharding: pure data parallel over the batch dim — core k gets batches
[2k, 2k+2).

Per-core design (memory-bound, ~67 MB HBM traffic/core, ~190 us roofline):

Oct-row layout: a group of G=4 channel planes (1 MB, contiguous in DRAM) is
loaded so SBUF partition p = (c*32 + row//8) holds 8 consecutive rows — a
pure [128, 2048] reshape of the DRAM stream (8 KB contiguous runs, 2-dim
AP). Both Haar butterfly stages are then same-partition, unit/2-strided
VectorE ops:
  stage 1 (vertical):  sum/diff of row pairs  -> one sd tile [128,2,4,256]
  scale: ScalarE in-place *0.5 on sd (folds the Haar normalization)
  stage 2 (horizontal): sd even +/- odd cols  -> oadd [p,v,rh4,j] holds
       subbands 0 (v=0) and 1 (v=1); osub holds subbands 2 and 3.
Stores are [128, 512] -> 256 KB per subband pair slice with 2 KB contiguous
DRAM runs (2-dim AP, full 128 partitions): output row i = 4*(p%32) + rh4.

Engine budget/core: DVE 4 ops/group ~150 us, ACT ~66 us, DMA ~195-215 us
(bottleneck), PE unused (fp32 matmul runs at 1/4 rate — measured slower
than DVE for this transform).
"""

import sys

sys.path.insert(0, "/opt/trn_rl_repo")

import numpy as np

import concourse.bacc as bacc
import concourse.mybir as mybir
from concourse.tile import TileContext

N_CORES = 8
N_PER_CORE = 2  # batches per core
C = 64  # input channels
H = 256
W = 256
G = 8  # channels per group (2 MB loads, 16 rows/partition)
F32 = mybir.dt.float32


def build_nc():
    nc = bacc.Bacc("TRN2", target_bir_lowering=False, debug=False)
    x = nc.dram_tensor("x", [N_PER_CORE, C, H, W], F32, kind="ExternalInput")
    y = nc.dram_tensor("y", [N_PER_CORE, 4 * C, H // 2, W // 2], F32, kind="ExternalOutput")

    with TileContext(nc) as tc:
        with (
            tc.tile_pool(name="inpool", bufs=3) as inpool,
            tc.tile_pool(name="sdpool", bufs=3) as sdpool,
            tc.tile_pool(name="outpool", bufs=3) as outpool,
        ):
            gi = 0
            for n in range(N_PER_CORE):
                for c0 in range(0, C, G):
                    # --- load: pure reshape of the 1 MB contiguous group.
                    # it[p, o, w] = x[n, c0 + p//32, 8*(p%32) + o, w]
                    it = inpool.tile([128, G * 512], F32, tag="in")
                    src = x[n, c0 : c0 + G].rearrange("c (q o) w -> (c q) o w", o=2 * G)
                    nc.sync.dma_start(
                        out=it[:].rearrange("p (o w) -> p o w", o=2 * G), in_=src
                    )

                    # --- stage 1 (vertical): rows 2t / 2t+1 within a partition
                    itv = it[:].rearrange("p (r t w) -> p r t w", r=G, t=2)
                    sd = sdpool.tile([128, G * 512], F32, tag="sd")
                    sdv = sd[:].rearrange("p (v r w) -> p v r w", v=2, r=G)
                    nc.vector.tensor_add(
                        out=sdv[:, 0], in0=itv[:, :, 0, :], in1=itv[:, :, 1, :]
                    )
                    nc.vector.tensor_sub(
                        out=sdv[:, 1], in0=itv[:, :, 0, :], in1=itv[:, :, 1, :]
                    )

                    # --- Haar 0.5 normalization, in place on ScalarE
                    nc.scalar.mul(sd[:], sd[:], 0.5)

                    # --- stage 2 (horizontal): even/odd column butterfly
                    sdj = sd[:].rearrange("p (v r j t) -> p v r j t", v=2, r=G, t=2)
                    oadd = outpool.tile([128, G * 256], F32, tag="oadd")
                    osub = outpool.tile([128, G * 256], F32, tag="osub")
                    oadd_v = oadd[:].rearrange("p (v r j) -> p v r j", v=2, r=G)
                    osub_v = osub[:].rearrange("p (v r j) -> p v r j", v=2, r=G)
                    nc.vector.tensor_add(
                        out=oadd_v, in0=sdj[..., 0], in1=sdj[..., 1]
                    )
                    nc.vector.tensor_sub(
                        out=osub_v, in0=sdj[..., 0], in1=sdj[..., 1]
                    )

                    # --- stores: (tile, v, subband); i = 4*(p%32) + rh4, so the
                    # DRAM side is [p step 512][(r j) 512] — 2 KB runs.
                    for t_, v, s in ((oadd, 0, 0), (oadd, 1, 1), (osub, 0, 2), (osub, 1, 3)):
                        dst = y[n, s * C + c0 : s * C + c0 + G].rearrange(
                            "c (q r) j -> (c q) (r j)", r=G
                        )
                        eng = nc.sync if (gi * 4 + s) % 2 == 0 else nc.scalar
                        eng.dma_start(
                            out=dst,
                            in_=t_[:].rearrange("p (v f) -> p v f", v=2)[:, v],
                        )
                    gi += 1

    nc.finalize()
    return nc


_NC = None


def _get_nc():
    global _NC
    if _NC is None:
        _NC = build_nc()
    return _NC


def kernel(x: np.ndarray) -> np.ndarray:
    from concourse.bass_utils import run_bass_kernel_spmd

    x = np.ascontiguousarray(np.asarray(x), dtype=np.float32)
    assert x.shape == (16, C, H, W), x.shape

    nc = _get_nc()
    in_maps = [
        {"x": x[k * N_PER_CORE : (k + 1) * N_PER_CORE]} for k in range(N_CORES)
    ]
    res = run_bass_kernel_spmd(nc, in_maps, core_ids=list(range(N_CORES)))
    return np.concatenate([r["y"] for r in res.results], axis=0)


# revision 6
# speedup vs baseline: 1.1452x; 1.0367x over previous
"""2D Haar DWT (level 1) Trainium2 Bass kernel.

Input  x: [16, 64, 256, 256] f32
Output y: [16, 256, 128, 128] f32, y[n, s*64+c, i, j] = Haar mix s of the
2x2 block x[n, c, 2i:2i+2, 2j:2j+2].

Sharding: pure data parallel over the batch dim — core k gets batches
[2k, 2k+2).

Per-core design (memory-bound, ~67 MB HBM traffic/core, ~190 us roofline):

Oct-row layout: a group of G=4 channel planes (1 MB, contiguous in DRAM) is
loaded so SBUF partition p = (c*32 + row//8) holds 8 consecutive rows — a
pure [128, 2048] reshape of the DRAM stream (8 KB contiguous runs, 2-dim
AP). Both Haar butterfly stages are then same-partition, unit/2-strided
VectorE ops:
  stage 1 (vertical):  sum/diff of row pairs  -> one sd tile [128,2,4,256]
  scale: ScalarE in-place *0.5 on sd (folds the Haar normalization)
  stage 2 (horizontal): sd even +/- odd cols  -> oadd [p,v,rh4,j] holds
       subbands 0 (v=0) and 1 (v=1); osub holds subbands 2 and 3.
Stores are [128, 512] -> 256 KB per subband pair slice with 2 KB contiguous
DRAM runs (2-dim AP, full 128 partitions): output row i = 4*(p%32) + rh4.

Engine budget/core: DVE 4 ops/group ~150 us, ACT ~66 us, DMA ~195-215 us
(bottleneck), PE unused (fp32 matmul runs at 1/4 rate — measured slower
than DVE for this transform).
"""

import sys

sys.path.insert(0, "/opt/trn_rl_repo")

import numpy as np

import concourse.bacc as bacc
import concourse.mybir as mybir
from concourse.tile import TileContext

N_CORES = 8
N_PER_CORE = 2  # batches per core
C = 64  # input channels
H = 256
W = 256
G = 8  # channels per group (2 MB loads, 16 rows/partition)
F32 = mybir.dt.float32


def build_nc():
    nc = bacc.Bacc("TRN2", target_bir_lowering=False, debug=False)
    x = nc.dram_tensor("x", [N_PER_CORE, C, H, W], F32, kind="ExternalInput")
    y = nc.dram_tensor("y", [N_PER_CORE, 4 * C, H // 2, W // 2], F32, kind="ExternalOutput")

    with TileContext(nc) as tc:
        with (
            tc.tile_pool(name="inpool", bufs=3) as inpool,
            tc.tile_pool(name="sdpool", bufs=3) as sdpool,
            tc.tile_pool(name="outpool", bufs=4) as outpool,
        ):
            gi = 0
            for n in range(N_PER_CORE):
                for c0 in range(0, C, G):
                    # --- load: pure reshape of the 1 MB contiguous group.
                    # it[p, o, w] = x[n, c0 + p//32, 8*(p%32) + o, w]
                    it = inpool.tile([128, G * 512], F32, tag="in")
                    src = x[n, c0 : c0 + G].rearrange("c (q o) w -> (c q) o w", o=2 * G)
                    nc.sync.dma_start(
                        out=it[:].rearrange("p (o w) -> p o w", o=2 * G), in_=src
                    )

                    # --- stage 1 (vertical): rows 2t / 2t+1 within a partition
                    itv = it[:].rearrange("p (r t w) -> p r t w", r=G, t=2)
                    sd = sdpool.tile([128, G * 512], F32, tag="sd")
                    sdv = sd[:].rearrange("p (v r w) -> p v r w", v=2, r=G)
                    nc.vector.tensor_add(
                        out=sdv[:, 0], in0=itv[:, :, 0, :], in1=itv[:, :, 1, :]
                    )
                    nc.vector.tensor_sub(
                        out=sdv[:, 1], in0=itv[:, :, 0, :], in1=itv[:, :, 1, :]
                    )

                    # --- Haar 0.5 normalization, in place on ScalarE
                    nc.scalar.mul(sd[:], sd[:], 0.5)

                    # --- stage 2 (horizontal): even/odd column butterfly
                    sdj = sd[:].rearrange("p (v r j t) -> p v r j t", v=2, r=G, t=2)
                    oadd = outpool.tile([128, G * 256], F32, tag="oadd")
                    osub = outpool.tile([128, G * 256], F32, tag="osub")
                    oadd_v = oadd[:].rearrange("p (v r j) -> p v r j", v=2, r=G)
                    osub_v = osub[:].rearrange("p (v r j) -> p v r j", v=2, r=G)
                    nc.vector.tensor_add(
                        out=oadd_v, in0=sdj[..., 0], in1=sdj[..., 1]
                    )
                    nc.vector.tensor_sub(
                        out=osub_v, in0=sdj[..., 0], in1=sdj[..., 1]
                    )

                    # --- stores: (tile, v, subband); i = 4*(p%32) + rh4, so the
                    # DRAM side is [p step 512][(r j) 512] — 2 KB runs.
                    for t_, v, s in ((oadd, 0, 0), (oadd, 1, 1), (osub, 0, 2), (osub, 1, 3)):
                        dst = y[n, s * C + c0 : s * C + c0 + G].rearrange(
                            "c (q r) j -> (c q) (r j)", r=G
                        )
                        eng = nc.sync if (gi * 4 + s) % 2 == 0 else nc.scalar
                        eng.dma_start(
                            out=dst,
                            in_=t_[:].rearrange("p (v f) -> p v f", v=2)[:, v],
                        )
                    gi += 1

    nc.finalize()
    return nc


_NC = None


def _get_nc():
    global _NC
    if _NC is None:
        _NC = build_nc()
    return _NC


def kernel(x: np.ndarray) -> np.ndarray:
    from concourse.bass_utils import run_bass_kernel_spmd

    x = np.ascontiguousarray(np.asarray(x), dtype=np.float32)
    assert x.shape == (16, C, H, W), x.shape

    nc = _get_nc()
    in_maps = [
        {"x": x[k * N_PER_CORE : (k + 1) * N_PER_CORE]} for k in range(N_CORES)
    ]
    res = run_bass_kernel_spmd(nc, in_maps, core_ids=list(range(N_CORES)))
    return np.concatenate([r["y"] for r in res.results], axis=0)
